# revision 1
# baseline (speedup 1.0000x reference)
"""Trainium2 Bass kernel for ComplexGCN (3x GCNConv + 2x MHA), 8-core SPMD.

Strategy: shard destination nodes across 8 cores (512 nodes/core). Each core
builds its dense normalized-adjacency shard A^T [4096 src, 512 dest] in bf16
on-device from the (host-sorted) edge list via iota/is_equal one-hot matmuls,
then every layer's message passing is a dense matmul with A^T stationary.
Attention is computed in transposed (feature-major) layout with ACT exp and a
ones-column in V for the softmax denominator. Cross-core: AllGather of degree
vector and of the (bf16) node-feature matrices between stages.

Host-side work is limited to index manipulation / layout (sort, pad,
transpose, concat); all floating-point math on input values happens on-device.
"""

import numpy as np

import concourse.bass as bass
import concourse.bacc as bacc
import concourse.mybir as mybir
import concourse.tile as tile
from concourse import bass_utils
from concourse.masks import make_identity

P = 128
N = 4096
NCORES = 8
NPC = N // NCORES          # 512 nodes per core
NSTRIP = NPC // P          # 4 dest strips per core
NST = N // P               # 32 src tiles
DIN = 256
HID = 256
DOUT = 128
NH = 4
DH = 64

f32 = mybir.dt.float32
bf16 = mybir.dt.bfloat16
AF = mybir.ActivationFunctionType
ALU = mybir.AluOpType
RG = [list(range(NCORES))]


# ----------------------------------------------------------------------------
# Host-side prep: pure index manipulation + layout.
# ----------------------------------------------------------------------------

def _prep_edges(edge_index, edge_weight):
    """Partition/sort/pad edges per core into fixed chunk cells.

    Returns (M, cell_off, erow, ecol, eww):
      M[s, t]      chunks for cell (dest strip s, src tile t), same all cores
      cell_off[s,t] starting chunk column of the cell
      erow/ecol    [NCORES, 128, C] fp32 relative ids (pad -1)
      eww          [NCORES, 128, C] fp32 edge weights (pad 0)
    """
    rows = np.concatenate([np.asarray(edge_index[0], np.int64),
                           np.arange(N, dtype=np.int64)])
    cols = np.concatenate([np.asarray(edge_index[1], np.int64),
                           np.arange(N, dtype=np.int64)])
    w = np.concatenate([np.asarray(edge_weight, np.float32),
                        np.ones(N, np.float32)])

    core = cols // NPC
    strip = (cols % NPC) // P
    stile = rows // P
    cell = (core * NSTRIP + strip) * NST + stile
    order = np.argsort(cell, kind="stable")
    srows, scols, sw, scell = rows[order], cols[order], w[order], cell[order]

    cnt = np.bincount(cell, minlength=NCORES * NSTRIP * NST)
    cnt = cnt.reshape(NCORES, NSTRIP, NST)
    M = (-((-cnt) // P)).max(axis=0)                  # ceil, max over cores
    C = int(M.sum())
    cell_off = np.zeros((NSTRIP, NST), np.int64)
    off = 0
    for s in range(NSTRIP):
        for t in range(NST):
            cell_off[s, t] = off
            off += M[s, t]

    erow = np.full((NCORES, P, C), -1.0, np.float32)
    ecol = np.full((NCORES, P, C), -1.0, np.float32)
    eww = np.zeros((NCORES, P, C), np.float32)
    starts = np.searchsorted(scell, np.arange(NCORES * NSTRIP * NST + 1))
    for c in range(NCORES):
        for s in range(NSTRIP):
            for t in range(NST):
                k = (c * NSTRIP + s) * NST + t
                a, b = int(starts[k]), int(starts[k + 1])
                n = b - a
                if n == 0:
                    continue
                m = int(M[s, t])
                o = int(cell_off[s, t])
                rr = np.full(m * P, -1.0, np.float32)
                cc = np.full(m * P, -1.0, np.float32)
                ww = np.zeros(m * P, np.float32)
                rr[:n] = (srows[a:b] % P).astype(np.float32)
                cc[:n] = (scols[a:b] % P).astype(np.float32)
                ww[:n] = sw[a:b]
                erow[c, :, o:o + m] = rr.reshape(m, P).T
                ecol[c, :, o:o + m] = cc.reshape(m, P).T
                eww[c, :, o:o + m] = ww.reshape(m, P).T
    return M, cell_off, erow, ecol, eww


# ----------------------------------------------------------------------------
# Device program
# ----------------------------------------------------------------------------

def _build_program(M, cell_off):
    C = int(M.sum())
    nc = bacc.Bacc("TRN2", target_bir_lowering=False, debug=False,
                   num_devices=NCORES)

    # ---- external I/O ----
    d_x0T = nc.dram_tensor("x0T", [DIN, N], f32, kind="ExternalInput")
    d_W1 = nc.dram_tensor("W1", [DIN, HID], f32, kind="ExternalInput")
    d_W2b = nc.dram_tensor("W2b", [HID + 1, HID], f32, kind="ExternalInput")
    d_W3b = nc.dram_tensor("W3b", [HID + 1, DOUT], f32, kind="ExternalInput")
    d_ipwT = nc.dram_tensor("ipwT", [HID, 3 * HID], f32, kind="ExternalInput")
    d_ipbC = nc.dram_tensor("ipbC", [P, 6], f32, kind="ExternalInput")
    d_opwTb = nc.dram_tensor("opwTb", [HID + 1, HID], f32, kind="ExternalInput")
    d_b1bc = nc.dram_tensor("b1bc", [P, HID], f32, kind="ExternalInput")
    d_bvbc = nc.dram_tensor("bvbc", [P, HID], f32, kind="ExternalInput")
    d_edat = nc.dram_tensor("edat", [P, 3 * C], f32, kind="ExternalInput")
    d_out = nc.dram_tensor("out", [NPC, DOUT], f32, kind="ExternalOutput")

    # ---- internal DRAM for collectives ----
    d_degl = nc.dram_tensor("deg_loc", [NPC], f32)
    d_degg = nc.dram_tensor("deg_glob", [N], f32, addr_space="Shared")
    ag_bufs = []
    for i, (shape_l, shape_g) in enumerate([
        ([HID, NPC], [NCORES, HID, NPC]),      # x1T
        ([NPC, HID], [NCORES, NPC, HID]),      # x2
        ([HID, NPC], [NCORES, HID, NPC]),      # x3T
        ([NPC, HID], [NCORES, NPC, HID]),      # x4
    ]):
        loc = nc.dram_tensor(f"ag{i}_loc", shape_l, bf16)
        glob = nc.dram_tensor(f"ag{i}_glob", shape_g, bf16, addr_space="Shared")
        ag_bufs.append((loc, glob))

    with tile.TileContext(nc) as tc:
        _emit(nc, tc, M, cell_off, C,
              d_x0T, d_W1, d_W2b, d_W3b, d_ipwT, d_ipbC, d_opwTb,
              d_b1bc, d_bvbc, d_edat, d_out,
              d_degl, d_degg, ag_bufs)
    nc.compile()
    return nc


def _emit(nc, tc, M, cell_off, C,
          d_x0T, d_W1, d_W2b, d_W3b, d_ipwT, d_ipbC, d_opwTb,
          d_b1bc, d_bvbc, d_edat, d_out,
          d_degl, d_degg, ag_bufs):
    from contextlib import ExitStack
    ctx = ExitStack()
    with ctx:
        const = ctx.enter_context(tc.tile_pool(name="const", bufs=1))
        big = ctx.enter_context(tc.tile_pool(name="big", bufs=1))
        scr = ctx.enter_context(tc.tile_pool(name="scr", bufs=2))
        ohp = ctx.enter_context(tc.tile_pool(name="ohp", bufs=6))
        exp_p = ctx.enter_context(tc.tile_pool(name="exp_p", bufs=4))
        tmp = ctx.enter_context(tc.tile_pool(name="tmp", bufs=4))
        psum = ctx.enter_context(tc.tile_pool(name="psum", bufs=2, space="PSUM"))
        psA = psB = psC = psum

        # ---------------- constants ----------------
        iota_i = const.tile([P, P], mybir.dt.int32, name="iota_i")
        nc.gpsimd.iota(iota_i[:], pattern=[[1, P]], base=0, channel_multiplier=0)
        iota_bf = const.tile([P, P], bf16, name="iota_bf")
        nc.vector.tensor_copy(iota_bf[:], iota_i[:])
        ident = const.tile([P, P], bf16, name="ident")
        make_identity(nc, ident[:])
        ones_col = const.tile([P, 1], bf16, name="ones_col")
        nc.vector.memset(ones_col[:], 1.0)
        ones_row = const.tile([1, NPC], bf16, name="ones_row")
        ones64f = const.tile([1, DH], f32, name="ones64f")
        nc.vector.memset(ones64f[:], 1.0)
        nc.vector.memset(ones_row[:], 1.0)

        # edge chunk data (fp32 scalar sources), one tensor -> one DMA/sem
        edat_sb = const.tile([P, 3 * C], f32, name="edat_sb")
        nc.sync.dma_start(edat_sb[:], d_edat[:, :])
        erow_sb = edat_sb[:, 0:C]
        ecol_sb = edat_sb[:, C:2 * C]
        eww_sb = edat_sb[:, 2 * C:3 * C]

        # biases
        ipbC = const.tile([P, 6], f32, name="ipbC")
        nc.sync.dma_start(ipbC[:], d_ipbC[:, :])
        b1bc = const.tile([P, HID], f32, name="b1bc")
        nc.sync.dma_start(b1bc[:], d_b1bc[:, :])
        bvbc = const.tile([P, HID], f32, name="bvbc")
        nc.sync.dma_start(bvbc[:], d_bvbc[:, :])

        def load_bf16(dram, rows, cols, tag):
            """DMA fp32 [rows<=128, cols] from dram AP + convert to bf16."""
            t_f = scr.tile([P, cols], f32, name="ldf32")
            nc.sync.dma_start(t_f[:rows, :], dram)
            t_b = const.tile([rows, cols], bf16, name=tag)
            nc.vector.tensor_copy(t_b[:], t_f[:rows, :])
            return t_b

        W1b = [load_bf16(d_W1[k * P:(k + 1) * P, :], P, HID, f"W1b{k}")
               for k in range(2)]
        W2b = [load_bf16(d_W2b[k * P:(k + 1) * P, :], P, HID, f"W2b{k}")
               for k in range(2)]
        W2b.append(load_bf16(d_W2b[2 * P:2 * P + 1, :], 1, HID, "W2b2"))
        W3b = [load_bf16(d_W3b[k * P:(k + 1) * P, :], P, DOUT, f"W3b{k}")
               for k in range(2)]
        W3b.append(load_bf16(d_W3b[2 * P:2 * P + 1, :], 1, DOUT, "W3b2"))
        ipwT = [load_bf16(d_ipwT[k * P:(k + 1) * P, :], P, 3 * HID, f"ipwT{k}")
                for k in range(2)]
        opwTb = [load_bf16(d_opwTb[k * P:(k + 1) * P, :], P, HID, f"opwTb{k}")
                 for k in range(2)]
        opwTb.append(load_bf16(d_opwTb[2 * P:2 * P + 1, :], 1, HID, "opwTb2"))

        # x0T fp32 -> bf16 [2][128, N]
        x0T = []
        for k in range(2):
            xb = big.tile([P, N], bf16, name=f"x0T{k}")
            for h in range(2):
                sl = slice(h * (N // 2), (h + 1) * (N // 2))
                t_f = scr.tile([P, N // 2], f32, name="x0scr")
                nc.sync.dma_start(t_f[:], d_x0T[k * P:(k + 1) * P, sl])
                nc.vector.tensor_copy(xb[:, sl], t_f[:])
            x0T.append(xb)

        # ---------------- persistent big tiles ----------------
        AT = [big.tile([P, NPC], bf16, name=f"AT{t}") for t in range(NST)]
        h1 = [big.tile([P, HID], bf16, name=f"h1_{m}") for m in range(NST)]
        xT_full = [big.tile([P, N], bf16, name=f"xTf{k}") for k in range(2)]
        xT_own = [big.tile([P, NPC], bf16, name=f"xTo{k}") for k in range(2)]
        xN_full = [big.tile([P, HID], bf16, name=f"xNf{m}") for m in range(NST)]
        kT = [big.tile([P, N], bf16, name=f"kT{g}") for g in range(2)]
        qT = [big.tile([P, NPC], bf16, name=f"qT{g}") for g in range(2)]
        v_aug = [big.tile([P, NH * (DH + 1)], bf16, name=f"vaug{m}")
                 for m in range(NST)]
        attnT = [big.tile([P, NPC], bf16, name=f"attnT{g}") for g in range(2)]
        x_n = [big.tile([P, HID], bf16, name=f"x_n{m}") for m in range(NSTRIP)]
        agg_s = [big.tile([P, HID], bf16, name=f"agg{m}") for m in range(NSTRIP)]
        aggT = [big.tile([P, NPC], bf16, name=f"aggT{k}") for k in range(2)]

        # ---------------- phase 1: build unnormalized A^T ----------------
        for s in range(NSTRIP):
            for t in range(NST):
                m = int(M[s, t])
                dst = AT[t][:, s * P:(s + 1) * P]
                if m == 0:
                    nc.vector.memset(dst, 0.0)
                    continue
                pA = psA.tile([P, P], f32, name="ps_mm")
                for j in range(m):
                    o = int(cell_off[s, t]) + j
                    roh = ohp.tile([P, P], bf16, name="roh")
                    coh = ohp.tile([P, P], bf16, name="coh")
                    nc.vector.tensor_scalar(
                        roh[:], iota_bf[:], erow_sb[:, o:o + 1],
                        eww_sb[:, o:o + 1], op0=ALU.is_equal, op1=ALU.mult)
                    nc.vector.tensor_scalar(
                        coh[:], iota_bf[:], ecol_sb[:, o:o + 1], None,
                        op0=ALU.is_equal)
                    nc.tensor.matmul(pA[:], lhsT=roh[:], rhs=coh[:],
                                     start=(j == 0), stop=(j == m - 1))
                nc.scalar.copy(dst, pA[:])

        # ---------------- phase 2: deg -> dinv; scale A^T ----------------
        deg_own = const.tile([P, NSTRIP], f32, name="deg_own")
        for s in range(NSTRIP):
            pd = psB.tile([P, 1], f32, name="ps_sm")
            for t in range(NST):
                nc.tensor.matmul(pd[:], lhsT=AT[t][:, s * P:(s + 1) * P],
                                 rhs=ones_col[:], start=(t == 0),
                                 stop=(t == NST - 1))
            nc.scalar.copy(deg_own[:, s:s + 1], pd[:])
        dinv_own = const.tile([P, NSTRIP], f32, name="dinv_own")
        nc.scalar.sqrt(dinv_own[:], deg_own[:])
        nc.vector.reciprocal(dinv_own[:], dinv_own[:])

        nc.sync.dma_start(
            d_degl.ap().rearrange("(m p) -> p m", p=P), deg_own[:])
        nc.gpsimd.collective_compute(
            "AllGather", ALU.bypass, replica_groups=RG,
            ins=[d_degl[:]], outs=[d_degg[:]])
        deg_all = const.tile([P, NST], f32, name="deg_all")
        nc.sync.dma_start(deg_all[:],
                          d_degg.ap().rearrange("(t p) -> p t", p=P))
        dinv_all = const.tile([P, NST], f32, name="dinv_all")
        nc.scalar.sqrt(dinv_all[:], deg_all[:])
        nc.vector.reciprocal(dinv_all[:], dinv_all[:])
        for t in range(NST):
            for s in range(NSTRIP):
                sl = AT[t][:, s * P:(s + 1) * P]
                nc.scalar.mul(sl, sl, dinv_all[:, t:t + 1])

        # ---------------- helpers ----------------
        def transpose_128(dst_ap, src_ap):
            pT = psC.tile([P, P], bf16, name="ps_sm")
            nc.tensor.transpose(pT[:], src_ap, ident[:])
            nc.scalar.copy(dst_ap, pT[:])

        def aggregate(rhs_tiles, width, out_tiles, bias_bc=None):
            """out[mm] = ACT(dinv_own[mm] * (sum_t AT[t](slice mm) @ rhs[t]))."""
            for mm in range(NSTRIP):
                pg = psB.tile([P, width], f32, name="ps_mm")
                for t in range(NST):
                    nc.tensor.matmul(pg[:], lhsT=AT[t][:, mm * P:(mm + 1) * P],
                                     rhs=rhs_tiles[t][:, :width],
                                     start=(t == 0), stop=(t == NST - 1))
                nc.scalar.mul(out_tiles[mm][:, :width], pg[:],
                              dinv_own[:, mm:mm + 1])
                if bias_bc is not None:
                    nc.vector.tensor_tensor(out_tiles[mm][:, :width],
                                            out_tiles[mm][:, :width],
                                            bias_bc[:, :width], op=ALU.add)

        def dense_out(lhsT_tiles, rhs3, width, evict):
            """For each dest tile: psum = sum_k lhsT[k].T @ rhs3[k] (+ ones-row
            K-aug for the bias), then evict(mm, psum_ap)."""
            for mm in range(NSTRIP):
                po = psum.tile([P, width], f32, name="ps_mm")
                for k in range(2):
                    nc.tensor.matmul(po[:], lhsT=lhsT_tiles[k][:, mm * P:(mm + 1) * P],
                                     rhs=rhs3[k][:, :width], start=(k == 0),
                                     stop=False)
                nc.tensor.matmul(po[:], lhsT=ones_row[0:1, mm * P:(mm + 1) * P],
                                 rhs=rhs3[2][:, :width], start=False, stop=True)
                evict(mm, po[:])

        def pre_ag_transpose(src_tiles):
            for mm in range(NSTRIP):
                for k in range(2):
                    transpose_128(xT_own[k][:, mm * P:(mm + 1) * P],
                                  src_tiles[mm][:, k * P:(k + 1) * P])

        def ag_fmajor(ag_idx):
            loc, glob = ag_bufs[ag_idx]
            for k in range(2):
                nc.sync.dma_start(loc[k * P:(k + 1) * P, :], xT_own[k][:])
            nc.gpsimd.collective_compute(
                "AllGather", ALU.bypass, replica_groups=RG,
                ins=[loc[:, :]], outs=[glob[:, :, :]])
            for c in range(NCORES):
                for k in range(2):
                    nc.sync.dma_start(
                        xT_full[k][:, c * NPC:(c + 1) * NPC],
                        glob[c, k * P:(k + 1) * P, :])

        def ag_nmajor(ag_idx, src_tiles):
            loc, glob = ag_bufs[ag_idx]
            for mm in range(NSTRIP):
                nc.sync.dma_start(loc[mm * P:(mm + 1) * P, :], src_tiles[mm][:])
            nc.gpsimd.collective_compute(
                "AllGather", ALU.bypass, replica_groups=RG,
                ins=[loc[:, :]], outs=[glob[:, :, :]])
            for c in range(NCORES):
                for mm in range(NSTRIP):
                    nc.sync.dma_start(xN_full[c * NSTRIP + mm][:],
                                      glob[c, mm * P:(mm + 1) * P, :])

        # ---------------- MHA ----------------
        def mha(out_tiles):
            # kT (all nodes), 2 head-groups
            for g in range(2):
                for n in range(NCORES):
                    pk = psB.tile([P, NPC], f32, name="ps_mm")
                    for k in range(2):
                        nc.tensor.matmul(
                            pk[:],
                            lhsT=ipwT[k][:, HID + g * P:HID + (g + 1) * P],
                            rhs=xT_full[k][:, n * NPC:(n + 1) * NPC],
                            start=(k == 0), stop=(k == 1))
                    nc.scalar.activation(kT[g][:, n * NPC:(n + 1) * NPC], pk[:],
                                         AF.Identity, bias=ipbC[:, 2 + g:3 + g])
            # qT (own nodes)
            for g in range(2):
                pq = psB.tile([P, NPC], f32, name="ps_mm")
                for k in range(2):
                    nc.tensor.matmul(pq[:], lhsT=ipwT[k][:, g * P:(g + 1) * P],
                                     rhs=xT_own[k][:], start=(k == 0),
                                     stop=(k == 1))
                nc.scalar.activation(qT[g][:], pq[:], AF.Identity,
                                     bias=ipbC[:, g:g + 1])
            # v (n-major, all nodes), interleaved [v_h | 1] blocks of 65
            for m in range(NST):
                pv = psB.tile([P, HID], f32, name="ps_mm")
                for k in range(2):
                    nc.tensor.matmul(pv[:], lhsT=xT_full[k][:, m * P:(m + 1) * P],
                                     rhs=ipwT[k][:, 2 * HID:3 * HID],
                                     start=(k == 0), stop=(k == 1))
                va = v_aug[m][:].rearrange("p (h x) -> p h x", x=DH + 1)
                nc.vector.tensor_tensor(
                    va[:, :, 0:DH],
                    pv[:].rearrange("p (h x) -> p h x", x=DH),
                    bvbc[:].rearrange("p (h x) -> p h x", x=DH), op=ALU.add)
                nc.vector.memset(va[:, :, DH:DH + 1], 1.0)
            # attention per head
            for h in range(NH):
                g, r = h // 2, (h % 2) * DH
                pat = psC.tile([DH + 1, NPC], f32, name="ps_at")
                for m in range(NST):
                    psc = psA.tile([P, NPC], f32, name="ps_sc")
                    nc.tensor.matmul(psc[:],
                                     lhsT=kT[g][r:r + DH, m * P:(m + 1) * P],
                                     rhs=qT[g][r:r + DH, :],
                                     start=True, stop=True)
                    et = exp_p.tile([P, NPC], bf16, name="expT")
                    nc.scalar.activation(et[:], psc[:], AF.Exp,
                                         scale=float(1.0 / np.sqrt(DH)))
                    nc.tensor.matmul(
                        pat[:], lhsT=v_aug[m][:, h * (DH + 1):(h + 1) * (DH + 1)],
                        rhs=et[:], start=(m == 0), stop=(m == NST - 1))
                rd = tmp.tile([1, NPC], f32, name="rd")
                nc.vector.reciprocal(rd[:], pat[DH:DH + 1, :])
                pb = psB.tile([DH, NPC], f32, name="ps_sm")
                nc.tensor.matmul(pb[:], lhsT=ones64f[0:1, :], rhs=rd[:],
                                 start=True, stop=True)
                rdb = tmp.tile([DH, NPC], f32, name="rdb")
                nc.scalar.copy(rdb[:], pb[:])
                nc.vector.tensor_tensor(attnT[g][r:r + DH, :], pat[0:DH, :],
                                        rdb[:], op=ALU.mult)
            # out-proj + relu -> out_tiles (n-major)
            dense_out(attnT, opwTb, HID,
                      lambda mm, ps: nc.scalar.activation(out_tiles[mm][:], ps,
                                                          AF.Relu))

        # ---------------- phase 3: GCN1 ----------------
        for m in range(NST):
            ph = psB.tile([P, HID], f32, name="ps_mm")
            for k in range(2):
                nc.tensor.matmul(ph[:], lhsT=x0T[k][:, m * P:(m + 1) * P],
                                 rhs=W1b[k][:], start=(k == 0), stop=(k == 1))
            nc.scalar.copy(h1[m][:], ph[:])
        aggregate(h1, HID, x_n, bias_bc=b1bc)
        pre_ag_transpose(x_n)
        ag_fmajor(0)

        # ---------------- phase 4: MHA1 -> x2 ----------------
        mha(x_n)
        ag_nmajor(1, x_n)

        # ---------------- phase 5: GCN2 ----------------
        aggregate(xN_full, HID, agg_s)
        for mm in range(NSTRIP):
            for k in range(2):
                transpose_128(aggT[k][:, mm * P:(mm + 1) * P],
                              agg_s[mm][:, k * P:(k + 1) * P])
        dense_out(aggT, W2b, HID,
                  lambda mm, ps: nc.scalar.copy(x_n[mm][:], ps))
        pre_ag_transpose(x_n)
        ag_fmajor(2)

        # ---------------- phase 6: MHA2 -> x4 ----------------
        mha(x_n)
        ag_nmajor(3, x_n)

        # ---------------- phase 7: GCN3 + sigmoid ----------------
        aggregate(xN_full, HID, agg_s)
        for mm in range(NSTRIP):
            for k in range(2):
                transpose_128(aggT[k][:, mm * P:(mm + 1) * P],
                              agg_s[mm][:, k * P:(k + 1) * P])
        def evict_sigmoid(mm, ps):
            o_sb = tmp.tile([P, DOUT], f32, name="o_sb")
            nc.scalar.activation(o_sb[:], ps, AF.Sigmoid)
            nc.sync.dma_start(d_out[mm * P:(mm + 1) * P, :], o_sb[:])
        dense_out(aggT, W3b, DOUT, evict_sigmoid)


# ----------------------------------------------------------------------------
# Entry point
# ----------------------------------------------------------------------------

_CACHE = {}
TRACE = False
LAST_RESULTS = None


def _get_program(M, cell_off):
    key = (M.tobytes(), cell_off.tobytes())
    if key not in _CACHE:
        _CACHE[key] = _build_program(M, cell_off)
    return _CACHE[key]


def make_in_maps(node_features, edge_index, edge_weight, W1, b1, W2, b2, W3,
                 b3, in_proj_w, in_proj_b, out_proj_w, out_proj_b):
    M, cell_off, erow, ecol, eww = _prep_edges(edge_index, edge_weight)
    asf = lambda a: np.ascontiguousarray(a, np.float32)
    common = {
        "x0T": asf(np.asarray(node_features, np.float32).T),
        "W1": asf(W1),
        "W2b": asf(np.vstack([W2, b2[None, :]])),
        "W3b": asf(np.vstack([W3, b3[None, :]])),
        "ipwT": asf(np.asarray(in_proj_w, np.float32).T),
        "ipbC": asf(np.asarray(in_proj_b, np.float32).reshape(6, P).T),
        "opwTb": asf(np.vstack([np.asarray(out_proj_w, np.float32).T,
                                out_proj_b[None, :]])),
        "b1bc": asf(np.broadcast_to(b1[None, :], (P, HID))),
        "bvbc": asf(np.broadcast_to(in_proj_b[None, 2 * HID:3 * HID],
                                    (P, HID))),
    }
    in_maps = []
    for c in range(NCORES):
        m = dict(common)
        m["edat"] = np.ascontiguousarray(
            np.concatenate([erow[c], ecol[c], eww[c]], axis=1))
        in_maps.append(m)
    return M, cell_off, in_maps


def kernel(**inputs):
    global LAST_RESULTS
    inputs = {k: np.asarray(v) for k, v in inputs.items()}
    M, cell_off, in_maps = make_in_maps(**inputs)
    nc = _get_program(M, cell_off)
    res = bass_utils.run_bass_kernel_spmd(nc, in_maps,
                                          core_ids=list(range(NCORES)),
                                          trace=TRACE)
    LAST_RESULTS = res
    out = np.concatenate([res.results[c]["out"] for c in range(NCORES)],
                         axis=0)
    return out.astype(np.float32)

